# revision 1
# baseline (speedup 1.0000x reference)
"""Trainium2 Bass kernel for nn_AttentionBlock (B=4, C=512, T=2048, H=8, G=32).

Sharding: 8 cores = (batch b in 0..3) x (head-group hg in 0..1, 4 heads each).
Each core computes groupnorm(x[b]) (redundantly within the pair), its heads'
q/k/v, attention, and a partial projection using its head-group's w_proj
columns.  Host sums the two partials per batch; the hg==0 core folds in the
residual x and b_proj.

On-chip layout highlights:
 - QK^T computed in scoresT[s, t] layout; two heads of a pair occupy
   partition halves 0-63 / 64-127 so their K=64 matmuls run concurrently on
   distinct PE row-groups.
 - exp via ACT psum->sbuf, no max subtraction (scores are O(5), safe in fp32).
 - V generated directly s-major (lhsT = h) and augmented with a ones column,
   so the PV matmul emits both out^T[d, t] and the softmax row sums.
 - softmax division: reciprocal of sums on DVE, broadcast across partitions
   via a K=1 ones matmul (exact fp32), then one DVE multiply.
 - big matmuls use fp32r (full-rate); tiny stat/broadcast matmuls use exact
   fp32.
"""

import sys
from contextlib import ExitStack

sys.path.insert(0, "/opt/trn_rl_repo")

import numpy as np

import concourse.bass as bass
import concourse.tile as tile
from concourse import bacc, mybir
from concourse.bass_utils import run_bass_kernel_spmd

F32 = mybir.dt.float32
F32R = mybir.dt.float32r
F16 = mybir.dt.float16
AF = mybir.ActivationFunctionType
OP = mybir.AluOpType

B, C, T = 4, 512, 2048
H = 8
DH = C // H          # 64
G = 32               # groupnorm groups
GS = C // G          # 16 channels per group
EPS = 1e-5
NKC = C // 128       # 4 c-chunks
NTC4 = T // 512      # 4 t-chunks of 512
SCALE = 1.0 / np.sqrt(np.sqrt(DH))

_CACHE = {}


def round_f32r(a):
    u = np.ascontiguousarray(a, np.float32).view(np.uint32)
    low = u & np.uint32(0xFFF)
    base = u & ~np.uint32(0xFFF)
    lsb = (base >> np.uint32(12)) & np.uint32(1)
    up = (low > 0x800) | ((low == 0x800) & (lsb == 1))
    out = base + (up.astype(np.uint32) << np.uint32(12))
    return out.view(np.float32)


def r(ap):
    return ap.bitcast(F32R)


def build_program():
    nc = bacc.Bacc("TRN2", target_bir_lowering=False, debug=False)

    def inp(name, shape, dt=F32):
        return nc.dram_tensor(name, shape, dt, kind="ExternalInput").ap()

    x_d = inp("x", [C, T])
    wqk_d = inp("wqk", [C, 512], F32R)
    wv_d = inp("wv", [C, 264], F32R)
    bv_d = inp("bv", [1, 264], F32R)
    wp_d = inp("wp", [256, 512], F32R)
    smalls_d = inp("smalls", [128, 25])
    expander_d = inp("expander", [8, 128])
    onest_d = inp("onest", [1, T], F32R)
    sel_d = inp("sel", [2, 128], F32R)
    y_d = nc.dram_tensor("y", [C, T], F32, kind="ExternalOutput").ap()

    x_r = x_d.rearrange("(k p) t -> k p t", p=128)
    wqk_r = wqk_d.rearrange("(k p) m -> k p m", p=128)
    wv_r = wv_d.rearrange("(k p) m -> k p m", p=128)
    wp_r = wp_d.rearrange("(k p) m -> k p m", p=128)
    y_r = y_d.rearrange("(k p) t -> k p t", p=128)

    with tile.TileContext(nc) as tc, ExitStack() as ctx:
        consts = ctx.enter_context(tc.tile_pool(name="consts", bufs=1))
        xpool = ctx.enter_context(tc.tile_pool(name="xpool", bufs=4))
        big = ctx.enter_context(tc.tile_pool(name="big", bufs=4))
        qkpool = ctx.enter_context(tc.tile_pool(name="qkpool", bufs=4))
        vtpool = ctx.enter_context(tc.tile_pool(name="vtpool", bufs=16))
        ptpool = ctx.enter_context(tc.tile_pool(name="ptpool", bufs=5))
        ysb = ctx.enter_context(tc.tile_pool(name="ysb", bufs=6))
        small = ctx.enter_context(tc.tile_pool(name="small", bufs=10))
        small2 = ctx.enter_context(tc.tile_pool(name="small2", bufs=1))
        stgp = ctx.enter_context(tc.tile_pool(name="stgp", bufs=2))
        divp = ctx.enter_context(tc.tile_pool(name="divp", bufs=2))
        pp_sc = ctx.enter_context(tc.tile_pool(name="pp_sc", bufs=3, space="PSUM"))
        pp_out = ctx.enter_context(tc.tile_pool(name="pp_out", bufs=2, space="PSUM"))

        ctr = [0]

        def psum_sc():
            ctr[0] += 1
            return pp_sc.tile([128, 1024], F32, tag="sc", name=f"sc{ctr[0]}")

        def psum_out(width=512):
            ctr[0] += 1
            return pp_out.tile([128, width], F32, tag="po", name=f"po{ctr[0]}")

        # ---- load x first (gn critical path); alternate DMA dispatchers ----
        xs = []
        for kc in range(NKC):
            x_t = xpool.tile([128, T], F32, tag="x", name=f"x{kc}")
            for j in range(4):
                eng = nc.sync if (kc * 4 + j) % 2 == 0 else nc.gpsimd
                eng.dma_start(
                    out=x_t[:, j * 512 : (j + 1) * 512],
                    in_=x_r[kc][:, j * 512 : (j + 1) * 512],
                )
            xs.append(x_t)

        # ---- load constants ----
        wqk_sb = consts.tile([128, NKC, 512], F32R)
        nc.gpsimd.dma_start(
            out=wqk_sb, in_=wqk_d.rearrange("(k p) m -> p k m", p=128)
        )
        wv_sb = consts.tile([128, NKC, 264], F32R)
        nc.sync.dma_start(
            out=wv_sb, in_=wv_d.rearrange("(k p) m -> p k m", p=128)
        )
        wp_sb = consts.tile([128, 2, 512], F32R)
        nc.gpsimd.dma_start(
            out=wp_sb, in_=wp_d.rearrange("(k p) m -> p k m", p=128)
        )
        smalls_sb = consts.tile([128, 25], F32)
        nc.sync.dma_start(out=smalls_sb, in_=smalls_d)
        bqk_sb = smalls_sb[:, 0:4]
        bp_sb = smalls_sb[:, 4:8]
        rs_sb = smalls_sb[:, 8:9]
        gamma_sb = smalls_sb[:, 9:13]
        beta_sb = smalls_sb[:, 13:17]
        ones16_sb = smalls_sb[:, 17:25]
        bv_sb = consts.tile([1, 264], F32R)
        nc.gpsimd.dma_start(out=bv_sb, in_=bv_d)
        expander_sb = consts.tile([8, 128], F32)
        nc.gpsimd.dma_start(out=expander_sb, in_=expander_d)
        onest_sb = consts.tile([1, T], F32R)
        nc.sync.dma_start(out=onest_sb, in_=onest_d)
        eps_sb = consts.tile([128, 1], F32)
        nc.vector.memset(eps_sb, EPS)
        sel_sb = consts.tile([2, 128], F32R)
        nc.gpsimd.dma_start(out=sel_sb, in_=sel_d)

        # ---- groupnorm ----
        hs = []
        for kc in range(NKC):
            x_t = xs[kc]
            stats = small.tile([128, 4, 6], F32, tag="stats")
            for j in range(4):
                nc.vector.bn_stats(
                    out=stats[:, j, :], in_=x_t[:, j * 512 : (j + 1) * 512]
                )
            mv = small.tile([128, 2], F32, tag="mv")
            nc.vector.bn_aggr(out=mv, in_=stats)

            # pack rhs: col0 = mean_c, col1 = E[x^2]_c = var_c + mean_c^2
            pk = small.tile([128, 2], F32, tag="pk")
            nc.vector.tensor_copy(pk[:, 0:1], mv[:, 0:1])
            nc.vector.tensor_mul(pk[:, 1:2], mv[:, 0:1], mv[:, 0:1])
            nc.vector.tensor_add(pk[:, 1:2], pk[:, 1:2], mv[:, 1:2])

            ps_g = psum_out()
            nc.tensor.matmul(
                ps_g[0:8, 0:2], ones16_sb, pk, start=True, stop=True
            )
            # gm: col0 = mean_g, col1 = rstd_g
            gsum = small.tile([8, 2], F32, tag="gsum")
            nc.vector.tensor_copy(gsum, ps_g[0:8, 0:2])
            gm = small.tile([8, 2], F32, tag="gm")
            nc.vector.tensor_copy(gm[:, 0:1], gsum[:, 0:1])
            varg = small.tile([8, 1], F32, tag="varg")
            nc.vector.tensor_mul(varg, gsum[:, 0:1], gsum[:, 0:1])
            nc.vector.tensor_sub(varg, gsum[:, 1:2], varg)
            nc.scalar.activation(varg, varg, AF.Sqrt, bias=eps_sb[0:8, :])
            nc.vector.reciprocal(gm[:, 1:2], varg)

            ps_pc = psum_out()
            nc.tensor.matmul(
                ps_pc[0:128, 0:2], expander_sb, gm, start=True, stop=True
            )
            scale = small.tile([128, 1], F32, tag="scale")
            nc.vector.tensor_mul(scale, ps_pc[:, 1:2], gamma_sb[:, kc : kc + 1])
            nbias = small.tile([128, 1], F32, tag="nbias")
            nc.vector.tensor_mul(nbias, ps_pc[:, 0:1], scale)
            nc.vector.tensor_sub(nbias, beta_sb[:, kc : kc + 1], nbias)

            h_t = big.tile([128, T], F32, tag="big")
            nc.vector.tensor_scalar(
                out=r(h_t),
                in0=x_t,
                scalar1=scale,
                scalar2=nbias,
                op0=OP.mult,
                op1=OP.add,
            )
            hs.append(h_t)

        # ---- q/k generation: m-chunks [qP0, kP0, qP1, kP1] ----
        qk_tiles = []
        for mc in range(4):
            dest = qkpool.tile([128, T], F32, tag="qk")
            qk_tiles.append(dest)
            for tc2 in range(2):
                ps = psum_sc()
                for half in range(2):
                    t0 = (tc2 * 2 + half) * 512
                    for kc in range(NKC):
                        nc.tensor.matmul(
                            ps[:, half * 512 : half * 512 + 512],
                            r(wqk_sb[:, kc, mc * 128 : mc * 128 + 128]),
                            r(hs[kc][:, t0 : t0 + 512]),
                            start=(kc == 0),
                            stop=(kc == NKC - 1),
                        )
                nc.vector.tensor_scalar(
                    out=r(dest[:, tc2 * 1024 : tc2 * 1024 + 1024]),
                    in0=ps,
                    scalar1=bqk_sb[:, mc : mc + 1],
                    scalar2=None,
                    op0=OP.add,
                )
        qpair = [qk_tiles[0], qk_tiles[2]]
        kpair = [qk_tiles[1], qk_tiles[3]]

        # ---- v generation, s-major with ones column ----
        vts = []
        for sc in range(16):
            ps = psum_sc()
            for kc in range(NKC):
                nc.tensor.matmul(
                    ps[:, 0:264],
                    r(hs[kc][:, sc * 128 : sc * 128 + 128]),
                    r(wv_sb[:, kc, :]),
                    start=(kc == 0),
                    stop=False,
                )
            nc.tensor.matmul(
                ps[:, 0:264],
                r(onest_sb[0:1, sc * 128 : sc * 128 + 128]),
                r(bv_sb),
                start=False,
                stop=True,
            )
            vt = vtpool.tile([128, 4, 66], F32, tag="vt")
            nc.vector.tensor_copy(r(vt), ps[:, 0:264])
            vts.append(vt)

        def proj_tc(tc4):
            for mc in range(4):
                ps = psum_sc()
                for kc2 in range(2):
                    nc.tensor.matmul(
                        ps[:, 0:512],
                        r(wp_sb[:, kc2, mc * 128 : mc * 128 + 128]),
                        r(att[kc2][:, tc4 * 512 : tc4 * 512 + 512]),
                        start=(kc2 == 0),
                        stop=(kc2 == 1),
                    )
                xz = ysb.tile([128, 512], F32, tag="y")
                nc.gpsimd.tensor_scalar(
                    out=xz,
                    in0=xs[mc][:, tc4 * 512 : tc4 * 512 + 512],
                    scalar1=rs_sb,
                    scalar2=bp_sb[:, mc : mc + 1],
                    op0=OP.mult,
                    op1=OP.add,
                )
                yt = ysb.tile([128, 512], F32, tag="y")
                nc.vector.tensor_add(yt, ps[:, 0:512], xz)
                eng = nc.sync if (mc + tc4) % 2 == 0 else nc.gpsimd
                eng.dma_start(
                    out=y_r[mc][:, tc4 * 512 : tc4 * 512 + 512], in_=yt
                )

        # ---- attention: heads of a pair run on PE row-group halves, both
        # heads' scoresT chunks share one [128, 1024] psum tile (col halves)
        # so the full array stays active and the two QK matmuls overlap.
        # t-chunks of 512; PV is M=65 (ones-augmented V) per head. ----
        sums_q = [
            [
                small2.tile(
                    [2, 1024], F32, tag=f"sums{i}{h}", name=f"sums{i}{h}"
                )
                for h in range(2)
            ]
            for i in range(2)
        ]
        att = [big.tile([128, T], F32, tag="big", name=f"att{i}") for i in range(2)]

        def divide_tq(pr, tq):
            co = (tq % 2) * 512
            t0 = tq * 512
            sq = sums_q[pr][tq // 2]
            rbp = divp.tile([2, 512], F32, tag="rb", name=f"rb{pr}{tq}")
            scrp = divp.tile([2, 512], F32, tag="scr", name=f"scr{pr}{tq}")
            nc.vector.reciprocal_approx_accurate(
                out=rbp, in_=sq[:, co : co + 512], scratch=scrp
            )
            rb2p = divp.tile([2, 512], F32, tag="rb2", name=f"rb2{pr}{tq}")
            nc.vector.tensor_copy(r(rb2p), rbp)
            for hip in range(2):
                bc = psum_sc()
                nc.tensor.matmul(
                    bc[0:64, 0:512],
                    sel_sb[:, hip * 64 : hip * 64 + 64],
                    r(rb2p),
                    start=True,
                    stop=True,
                )
                a_slc = att[pr][hip * 64 : hip * 64 + 64, t0 : t0 + 512]
                nc.vector.tensor_mul(r(a_slc), r(a_slc), bc[0:64, 0:512])

        def emit_qk(pr, tq, sc):
            qp, kp = qpair[pr], kpair[pr]
            t0 = tq * 512
            ps = psum_sc()
            nc.tensor.matmul(
                ps[:, 0:512],
                r(kp[0:64, sc * 128 : sc * 128 + 128]),
                r(qp[0:64, t0 : t0 + 512]),
                start=True,
                stop=True,
            )
            nc.tensor.matmul(
                ps[:, 512:1024],
                r(kp[64:128, sc * 128 : sc * 128 + 128]),
                r(qp[64:128, t0 : t0 + 512]),
                start=True,
                stop=True,
            )
            return ps

        def emit_pv(pr, tq, sc, ps, outA, outB):
            pt_t = ptpool.tile([128, 1024], F32, tag="pt")
            nc.scalar.activation(r(pt_t), ps, AF.Exp)
            va = vts[sc][:, pr * 2 + 0, 0:65]
            vb = vts[sc][:, pr * 2 + 1, 0:65]
            nc.tensor.matmul(
                outA[0:65, 0:512],
                r(va),
                r(pt_t[:, 0:512]),
                start=(sc == 0),
                stop=(sc == 15),
            )
            nc.tensor.matmul(
                outB[0:65, 0:512],
                r(vb),
                r(pt_t[:, 512:1024]),
                start=(sc == 0),
                stop=(sc == 15),
            )

        def block_epilogue(pr, tq, outA, outB):
            t0 = tq * 512
            for hip, outp in ((0, outA), (1, outB)):
                co = (tq % 2) * 512
                nc.vector.tensor_copy(
                    r(att[pr][hip * 64 : hip * 64 + 64, t0 : t0 + 512]),
                    outp[0:64, :],
                )
                stg = stgp.tile(
                    [65, 512], F32, tag="stg", name=f"stg{pr}_{tq}_{hip}"
                )
                nc.vector.tensor_copy(stg[64:65, 0:512], outp[64:65, :])
                nc.sync.dma_start(
                    out=sums_q[pr][tq // 2][hip : hip + 1, co : co + 512],
                    in_=stg[64:65, 0:512],
                )

        # flattened attention iterations with one-deep QK lookahead so the
        # PE issues QK(i+1) while ACT runs exp(i) — keeps both engines
        # back-to-back.  Division quarters / projection chunks are spliced
        # between iterations once their inputs are long since staged.
        iters = [(pr, tq, sc) for pr in range(2) for tq in range(4) for sc in range(16)]
        after = {
            (0, 1, 8): lambda: divide_tq(0, 0),
            (0, 2, 8): lambda: divide_tq(0, 1),
            (0, 3, 8): lambda: divide_tq(0, 2),
            (1, 0, 8): lambda: divide_tq(0, 3),
            (1, 1, 8): lambda: divide_tq(1, 0),
            (1, 1, 12): lambda: proj_tc(0),
            (1, 2, 8): lambda: divide_tq(1, 1),
            (1, 2, 12): lambda: proj_tc(1),
            (1, 3, 8): lambda: divide_tq(1, 2),
            (1, 3, 12): lambda: proj_tc(2),
        }
        outs = {}
        ps_next = emit_qk(*iters[0])
        for i, (pr, tq, sc) in enumerate(iters):
            if sc == 0:
                outs[(pr, tq)] = (psum_out(), psum_out())
            ps_cur = ps_next
            if i + 1 < len(iters):
                ps_next = emit_qk(*iters[i + 1])
            outA, outB = outs[(pr, tq)]
            emit_pv(pr, tq, sc, ps_cur, outA, outB)
            if sc == 15:
                block_epilogue(pr, tq, outA, outB)
                del outs[(pr, tq)]
            hook = after.get((pr, tq, sc))
            if hook is not None:
                hook()
        divide_tq(1, 3)
        proj_tc(3)

    nc.compile()
    return nc


def _consts():
    expander = np.zeros((8, 128), np.float32)
    for g in range(8):
        expander[g, g * 16 : (g + 1) * 16] = 1.0
    onest = np.ones((1, T), np.float32)
    sel = np.zeros((2, 128), np.float32)
    for u in range(2):
        sel[u, u * 64 : (u + 1) * 64] = 1.0
    return expander, onest, sel


def _core_weights(hg, w_qkv, b_qkv, w_proj, b_proj, gn_gamma, gn_beta):
    heads = [4 * hg + i for i in range(4)]
    qrows, krows, vrows = [], [], []
    for h in heads:
        base = h * 3 * DH
        qrows.append(np.arange(base, base + DH))
        krows.append(np.arange(base + DH, base + 2 * DH))
        vrows.append(np.arange(base + 2 * DH, base + 3 * DH))
    # m-chunks: [qP0, kP0, qP1, kP1]; each pair chunk = [head_even | head_odd]
    qk_order = np.concatenate(
        [qrows[0], qrows[1], krows[0], krows[1], qrows[2], qrows[3], krows[2], krows[3]]
    )
    wqk = round_f32r(w_qkv[qk_order].T * SCALE)
    bqk = np.ascontiguousarray((b_qkv[qk_order] * SCALE).reshape(4, 128).T)
    wv = np.zeros((C, 264), np.float32)
    bv = np.zeros((1, 264), np.float32)
    for i, vr in enumerate(vrows):
        wv[:, i * 66 : i * 66 + 64] = w_qkv[vr].T
        bv[0, i * 66 : i * 66 + 64] = b_qkv[vr]
        bv[0, i * 66 + 64] = 1.0
    wv = round_f32r(wv)
    bv = round_f32r(bv)
    att_cols = np.concatenate([np.arange(h * DH, (h + 1) * DH) for h in heads])
    wp = round_f32r(w_proj[:, att_cols].T)
    if hg == 0:
        bp = np.ascontiguousarray(b_proj.reshape(4, 128).T)
        rs = np.ones((128, 1), np.float32)
    else:
        bp = np.zeros((128, 4), np.float32)
        rs = np.zeros((128, 1), np.float32)
    gamma = np.ascontiguousarray(gn_gamma.reshape(4, 128).T)
    beta = np.ascontiguousarray(gn_beta.reshape(4, 128).T)
    ones16 = np.zeros((128, 8), np.float32)
    for g in range(8):
        ones16[g * 16 : (g + 1) * 16, g] = 1.0 / GS
    smalls = np.concatenate([bqk, bp, rs, gamma, beta, ones16], axis=1)
    return dict(wqk=wqk, wv=wv, bv=bv, wp=wp, smalls=smalls)


def kernel(x, gn_gamma, gn_beta, w_qkv, b_qkv, w_proj, b_proj, _trace=False):
    x = np.asarray(x, np.float32)
    gn_gamma = np.asarray(gn_gamma, np.float32)
    gn_beta = np.asarray(gn_beta, np.float32)
    w_qkv = np.asarray(w_qkv, np.float32)
    b_qkv = np.asarray(b_qkv, np.float32)
    w_proj = np.asarray(w_proj, np.float32)
    b_proj = np.asarray(b_proj, np.float32)

    if "nc" not in _CACHE:
        _CACHE["nc"] = build_program()
    nc = _CACHE["nc"]

    expander, onest, sel = _consts()
    hg_consts = [
        _core_weights(hg, w_qkv, b_qkv, w_proj, b_proj, gn_gamma, gn_beta)
        for hg in range(2)
    ]
    in_maps = []
    for core in range(8):
        b, hg = core // 2, core % 2
        m = dict(hg_consts[hg])
        m["x"] = np.ascontiguousarray(x[b])
        m["expander"] = expander
        m["onest"] = onest
        m["sel"] = sel
        in_maps.append(m)

    res = run_bass_kernel_spmd(
        nc, in_maps, core_ids=list(range(8)), trace=_trace
    )
    y = np.empty((B, C, T), np.float32)
    for b in range(B):
        y[b] = res.results[2 * b]["y"] + res.results[2 * b + 1]["y"]
    if _trace:
        _CACHE["last_results"] = res
    return y



# revision 2
# speedup vs baseline: 1.2716x; 1.2716x over previous
"""Trainium2 Bass kernel for nn_AttentionBlock (B=4, C=512, T=2048, H=8, G=32).

Sharding: 8 cores = (batch b in 0..3) x (head-group hg in 0..1, 4 heads each).
Each core computes groupnorm(x[b]) (redundantly within the pair), its heads'
q/k/v, attention, and a partial projection using its head-group's w_proj
columns.  Host sums the two partials per batch; the hg==0 core folds in the
residual x and b_proj.

On-chip layout highlights:
 - all big matmuls use fp16 operands (fp32 streams the moving operand at
   ~2 cycles/element on the xbus; 2-byte dtypes stream at 1 — measured
   511ns vs ~215ns for N=512).  PSUM accumulation stays fp32.
 - QK^T computed in scoresT[s, t] layout; two heads of a pair occupy
   partition halves 0-63 / 64-127 so their K=64 matmuls run concurrently on
   distinct PE row-groups.
 - exp via ACT psum->sbuf, no max subtraction (scores are O(5), safe).
 - V generated s-major; a constant ones column (memset once per tile) is
   appended per head so the PV matmul emits both out^T[d, t] and the
   softmax row sums.
 - k-bias is dropped entirely (softmax-invariant once q keeps its bias:
   score = (q+bq)·(k+bk) differs from (q+bq)·k only by per-query consts).
 - v-bias is folded into the projection bias on the host:
   wp @ (att + bv) = wp @ att + (wp @ bv), exact for any b_qkv.
 - softmax division: reciprocal of sums on DVE, broadcast across partitions
   via a K=1/2 ones matmul, then one DVE multiply.
"""

import sys
from contextlib import ExitStack

sys.path.insert(0, "/opt/trn_rl_repo")

import numpy as np

import concourse.bass as bass
import concourse.tile as tile
from concourse import bacc, mybir
from concourse.bass_utils import run_bass_kernel_spmd

F32 = mybir.dt.float32
F16 = mybir.dt.float16
AF = mybir.ActivationFunctionType
OP = mybir.AluOpType

B, C, T = 4, 512, 2048
H = 8
DH = C // H          # 64
G = 32               # groupnorm groups
GS = C // G          # 16 channels per group
EPS = 1e-5
NKC = C // 128       # 4 c-chunks
SCALE = 1.0 / np.sqrt(np.sqrt(DH))

_CACHE = {}


def build_program():
    nc = bacc.Bacc("TRN2", target_bir_lowering=False, debug=False)

    def inp(name, shape, dt=F32):
        return nc.dram_tensor(name, shape, dt, kind="ExternalInput").ap()

    x_d = inp("x", [C, T])
    wqk_d = inp("wqk", [C, 512], F16)
    wv_d = inp("wv", [C, 256], F16)
    wp_d = inp("wp", [256, 512], F16)
    smalls_d = inp("smalls", [128, 25])
    expander_d = inp("expander", [8, 128])
    sel_d = inp("sel", [2, 128], F16)
    y_d = nc.dram_tensor("y", [C, T], F32, kind="ExternalOutput").ap()

    x_r = x_d.rearrange("(k p) t -> k p t", p=128)
    y_r = y_d.rearrange("(k p) t -> k p t", p=128)

    with tile.TileContext(nc) as tc, ExitStack() as ctx:
        consts = ctx.enter_context(tc.tile_pool(name="consts", bufs=1))
        xpool = ctx.enter_context(tc.tile_pool(name="xpool", bufs=4))
        big = ctx.enter_context(tc.tile_pool(name="big", bufs=4))
        qkpool = ctx.enter_context(tc.tile_pool(name="qkpool", bufs=4))
        vtpool = ctx.enter_context(tc.tile_pool(name="vtpool", bufs=16))
        ptpool = ctx.enter_context(tc.tile_pool(name="ptpool", bufs=5))
        ysb = ctx.enter_context(tc.tile_pool(name="ysb", bufs=6))
        small = ctx.enter_context(tc.tile_pool(name="small", bufs=10))
        small2 = ctx.enter_context(tc.tile_pool(name="small2", bufs=1))
        stgp = ctx.enter_context(tc.tile_pool(name="stgp", bufs=2))
        divp = ctx.enter_context(tc.tile_pool(name="divp", bufs=2))
        pp_sc = ctx.enter_context(tc.tile_pool(name="pp_sc", bufs=3, space="PSUM"))
        pp_out = ctx.enter_context(tc.tile_pool(name="pp_out", bufs=2, space="PSUM"))

        ctr = [0]

        def psum_sc():
            ctr[0] += 1
            return pp_sc.tile([128, 1024], F32, tag="sc", name=f"sc{ctr[0]}")

        def psum_out(width=512):
            ctr[0] += 1
            return pp_out.tile([128, width], F32, tag="po", name=f"po{ctr[0]}")

        # ---- load x first (gn critical path); alternate DMA dispatchers ----
        xs = []
        for kc in range(NKC):
            x_t = xpool.tile([128, T], F32, tag="x", name=f"x{kc}")
            for j in range(4):
                eng = nc.sync if (kc * 4 + j) % 2 == 0 else nc.gpsimd
                eng.dma_start(
                    out=x_t[:, j * 512 : (j + 1) * 512],
                    in_=x_r[kc][:, j * 512 : (j + 1) * 512],
                )
            xs.append(x_t)

        # ---- load constants ----
        wqk_sb = consts.tile([128, NKC, 512], F16)
        nc.gpsimd.dma_start(
            out=wqk_sb, in_=wqk_d.rearrange("(k p) m -> p k m", p=128)
        )
        wv_sb = consts.tile([128, NKC, 256], F16)
        nc.sync.dma_start(
            out=wv_sb, in_=wv_d.rearrange("(k p) m -> p k m", p=128)
        )
        wp_sb = consts.tile([128, 2, 512], F16)
        nc.gpsimd.dma_start(
            out=wp_sb, in_=wp_d.rearrange("(k p) m -> p k m", p=128)
        )
        smalls_sb = consts.tile([128, 25], F32)
        nc.sync.dma_start(out=smalls_sb, in_=smalls_d)
        bqk_sb = smalls_sb[:, 0:4]
        bp_sb = smalls_sb[:, 4:8]
        rs_sb = smalls_sb[:, 8:9]
        gamma_sb = smalls_sb[:, 9:13]
        beta_sb = smalls_sb[:, 13:17]
        ones16_sb = smalls_sb[:, 17:25]
        expander_sb = consts.tile([8, 128], F32)
        nc.gpsimd.dma_start(out=expander_sb, in_=expander_d)
        eps_sb = consts.tile([128, 1], F32)
        nc.vector.memset(eps_sb, EPS)
        sel_sb = consts.tile([2, 128], F16)
        nc.gpsimd.dma_start(out=sel_sb, in_=sel_d)

        # ---- groupnorm ----
        hs = []
        for kc in range(NKC):
            x_t = xs[kc]
            stats = small.tile([128, 4, 6], F32, tag="stats")
            for j in range(4):
                nc.vector.bn_stats(
                    out=stats[:, j, :], in_=x_t[:, j * 512 : (j + 1) * 512]
                )
            mv = small.tile([128, 2], F32, tag="mv")
            nc.vector.bn_aggr(out=mv, in_=stats)

            # pack rhs: col0 = mean_c, col1 = E[x^2]_c = var_c + mean_c^2
            pk = small.tile([128, 2], F32, tag="pk")
            nc.vector.tensor_copy(pk[:, 0:1], mv[:, 0:1])
            nc.vector.tensor_mul(pk[:, 1:2], mv[:, 0:1], mv[:, 0:1])
            nc.vector.tensor_add(pk[:, 1:2], pk[:, 1:2], mv[:, 1:2])

            ps_g = psum_out()
            nc.tensor.matmul(
                ps_g[0:8, 0:2], ones16_sb, pk, start=True, stop=True
            )
            # gm: col0 = mean_g, col1 = rstd_g
            gsum = small.tile([8, 2], F32, tag="gsum")
            nc.vector.tensor_copy(gsum, ps_g[0:8, 0:2])
            gm = small.tile([8, 2], F32, tag="gm")
            nc.vector.tensor_copy(gm[:, 0:1], gsum[:, 0:1])
            varg = small.tile([8, 1], F32, tag="varg")
            nc.vector.tensor_mul(varg, gsum[:, 0:1], gsum[:, 0:1])
            nc.vector.tensor_sub(varg, gsum[:, 1:2], varg)
            nc.scalar.activation(varg, varg, AF.Sqrt, bias=eps_sb[0:8, :])
            nc.vector.reciprocal(gm[:, 1:2], varg)

            ps_pc = psum_out()
            nc.tensor.matmul(
                ps_pc[0:128, 0:2], expander_sb, gm, start=True, stop=True
            )
            scale = small.tile([128, 1], F32, tag="scale")
            nc.vector.tensor_mul(scale, ps_pc[:, 1:2], gamma_sb[:, kc : kc + 1])
            nbias = small.tile([128, 1], F32, tag="nbias")
            nc.vector.tensor_mul(nbias, ps_pc[:, 0:1], scale)
            nc.vector.tensor_sub(nbias, beta_sb[:, kc : kc + 1], nbias)

            h_t = big.tile([128, T], F16, tag="big")
            nc.vector.tensor_scalar(
                out=h_t,
                in0=x_t,
                scalar1=scale,
                scalar2=nbias,
                op0=OP.mult,
                op1=OP.add,
            )
            hs.append(h_t)

        # ---- q/k generation: m-chunks [qP0, kP0, qP1, kP1] ----
        qk_tiles = []
        for mc in range(4):
            dest = qkpool.tile([128, T], F16, tag="qk")
            qk_tiles.append(dest)
            for tc2 in range(2):
                ps = psum_sc()
                for half in range(2):
                    t0 = (tc2 * 2 + half) * 512
                    for kc in range(NKC):
                        nc.tensor.matmul(
                            ps[:, half * 512 : half * 512 + 512],
                            wqk_sb[:, kc, mc * 128 : mc * 128 + 128],
                            hs[kc][:, t0 : t0 + 512],
                            start=(kc == 0),
                            stop=(kc == NKC - 1),
                        )
                if mc % 2 == 0:  # q chunks: add bias
                    nc.vector.tensor_scalar(
                        out=dest[:, tc2 * 1024 : tc2 * 1024 + 1024],
                        in0=ps,
                        scalar1=bqk_sb[:, mc : mc + 1],
                        scalar2=None,
                        op0=OP.add,
                    )
                else:  # k chunks: bias dropped (softmax-invariant)
                    nc.vector.tensor_copy(
                        dest[:, tc2 * 1024 : tc2 * 1024 + 1024], ps
                    )
        qpair = [qk_tiles[0], qk_tiles[2]]
        kpair = [qk_tiles[1], qk_tiles[3]]

        # ---- v generation, s-major; ones column memset per head block ----
        vts = []
        for sc in range(16):
            ps = psum_sc()
            for kc in range(NKC):
                nc.tensor.matmul(
                    ps[:, 0:256],
                    hs[kc][:, sc * 128 : sc * 128 + 128],
                    wv_sb[:, kc, :],
                    start=(kc == 0),
                    stop=(kc == NKC - 1),
                )
            vt = vtpool.tile([128, 4, 65], F16, tag="vt")
            nc.gpsimd.memset(vt[:, :, 64:65], 1.0)
            nc.vector.tensor_copy(
                vt[:, :, 0:64],
                ps[:, 0:256].rearrange("p (h d) -> p h d", d=64),
            )
            vts.append(vt)

        def proj_tc(tc4):
            for mc in range(4):
                ps = psum_sc()
                for kc2 in range(2):
                    nc.tensor.matmul(
                        ps[:, 0:512],
                        wp_sb[:, kc2, mc * 128 : mc * 128 + 128],
                        att[kc2][:, tc4 * 512 : tc4 * 512 + 512],
                        start=(kc2 == 0),
                        stop=(kc2 == 1),
                    )
                xz = ysb.tile([128, 512], F32, tag="y")
                nc.gpsimd.tensor_scalar(
                    out=xz,
                    in0=xs[mc][:, tc4 * 512 : tc4 * 512 + 512],
                    scalar1=rs_sb,
                    scalar2=bp_sb[:, mc : mc + 1],
                    op0=OP.mult,
                    op1=OP.add,
                )
                yt = ysb.tile([128, 512], F32, tag="y")
                nc.vector.tensor_add(yt, ps[:, 0:512], xz)
                eng = nc.sync if (mc + tc4) % 2 == 0 else nc.gpsimd
                eng.dma_start(
                    out=y_r[mc][:, tc4 * 512 : tc4 * 512 + 512], in_=yt
                )

        # ---- attention: heads of a pair run on PE row-group halves, both
        # heads' scoresT chunks share one [128, 1024] psum tile (col halves)
        # so the full array stays active and the two QK matmuls overlap.
        # t-chunks of 512; PV is M=65 (ones-augmented V) per head. ----
        sums_q = [
            [
                small2.tile(
                    [2, 1024], F32, tag=f"sums{i}{h}", name=f"sums{i}{h}"
                )
                for h in range(2)
            ]
            for i in range(2)
        ]
        att = [big.tile([128, T], F16, tag="big", name=f"att{i}") for i in range(2)]

        def divide_tq(pr, tq):
            co = (tq % 2) * 512
            t0 = tq * 512
            sq = sums_q[pr][tq // 2]
            rbp = divp.tile([2, 512], F32, tag="rb", name=f"rb{pr}{tq}")
            scrp = divp.tile([2, 512], F32, tag="scr", name=f"scr{pr}{tq}")
            nc.vector.reciprocal_approx_accurate(
                out=rbp, in_=sq[:, co : co + 512], scratch=scrp
            )
            rb2p = divp.tile([2, 512], F16, tag="rb2", name=f"rb2{pr}{tq}")
            nc.vector.tensor_copy(rb2p, rbp)
            for hip in range(2):
                bc = psum_sc()
                nc.tensor.matmul(
                    bc[0:64, 0:512],
                    sel_sb[:, hip * 64 : hip * 64 + 64],
                    rb2p,
                    start=True,
                    stop=True,
                )
                a_slc = att[pr][hip * 64 : hip * 64 + 64, t0 : t0 + 512]
                nc.vector.tensor_mul(a_slc, a_slc, bc[0:64, 0:512])

        def emit_qk(pr, tq, sc):
            qp, kp = qpair[pr], kpair[pr]
            t0 = tq * 512
            ps = psum_sc()
            nc.tensor.matmul(
                ps[:, 0:512],
                kp[0:64, sc * 128 : sc * 128 + 128],
                qp[0:64, t0 : t0 + 512],
                start=True,
                stop=True,
            )
            nc.tensor.matmul(
                ps[:, 512:1024],
                kp[64:128, sc * 128 : sc * 128 + 128],
                qp[64:128, t0 : t0 + 512],
                start=True,
                stop=True,
            )
            return ps

        def emit_pv(pr, tq, sc, ps, outA, outB):
            pt_t = ptpool.tile([128, 1024], F16, tag="pt")
            nc.scalar.activation(pt_t, ps, AF.Exp)
            va = vts[sc][:, pr * 2 + 0, 0:65]
            vb = vts[sc][:, pr * 2 + 1, 0:65]
            nc.tensor.matmul(
                outA[0:65, 0:512],
                va,
                pt_t[:, 0:512],
                start=(sc == 0),
                stop=(sc == 15),
            )
            nc.tensor.matmul(
                outB[0:65, 0:512],
                vb,
                pt_t[:, 512:1024],
                start=(sc == 0),
                stop=(sc == 15),
            )

        def block_epilogue(pr, tq, outA, outB):
            t0 = tq * 512
            for hip, outp in ((0, outA), (1, outB)):
                co = (tq % 2) * 512
                nc.vector.tensor_copy(
                    att[pr][hip * 64 : hip * 64 + 64, t0 : t0 + 512],
                    outp[0:64, :],
                )
                stg = stgp.tile(
                    [65, 512], F32, tag="stg", name=f"stg{pr}_{tq}_{hip}"
                )
                nc.vector.tensor_copy(stg[64:65, 0:512], outp[64:65, :])
                nc.sync.dma_start(
                    out=sums_q[pr][tq // 2][hip : hip + 1, co : co + 512],
                    in_=stg[64:65, 0:512],
                )

        # flattened attention iterations with one-deep QK lookahead so the
        # PE issues QK(i+1) while ACT runs exp(i) — keeps both engines
        # back-to-back.  Division quarters / projection chunks are spliced
        # between iterations once their inputs are long since staged.
        iters = [(pr, tq, sc) for pr in range(2) for tq in range(4) for sc in range(16)]
        after = {
            (0, 1, 8): lambda: divide_tq(0, 0),
            (0, 2, 8): lambda: divide_tq(0, 1),
            (0, 3, 8): lambda: divide_tq(0, 2),
            (1, 0, 8): lambda: divide_tq(0, 3),
            (1, 1, 8): lambda: divide_tq(1, 0),
            (1, 1, 12): lambda: proj_tc(0),
            (1, 2, 8): lambda: divide_tq(1, 1),
            (1, 2, 12): lambda: proj_tc(1),
            (1, 3, 8): lambda: divide_tq(1, 2),
            (1, 3, 12): lambda: proj_tc(2),
        }
        outs = {}
        ps_next = emit_qk(*iters[0])
        for i, (pr, tq, sc) in enumerate(iters):
            if sc == 0:
                outs[(pr, tq)] = (psum_out(), psum_out())
            ps_cur = ps_next
            if i + 1 < len(iters):
                ps_next = emit_qk(*iters[i + 1])
            outA, outB = outs[(pr, tq)]
            emit_pv(pr, tq, sc, ps_cur, outA, outB)
            if sc == 15:
                block_epilogue(pr, tq, outA, outB)
                del outs[(pr, tq)]
            hook = after.get((pr, tq, sc))
            if hook is not None:
                hook()
        divide_tq(1, 3)
        proj_tc(3)

    nc.compile()
    return nc


def _consts():
    expander = np.zeros((8, 128), np.float32)
    for g in range(8):
        expander[g, g * 16 : (g + 1) * 16] = 1.0
    sel = np.zeros((2, 128), np.float16)
    for u in range(2):
        sel[u, u * 64 : (u + 1) * 64] = 1.0
    return expander, sel


def _core_weights(hg, w_qkv, b_qkv, w_proj, b_proj, gn_gamma, gn_beta):
    heads = [4 * hg + i for i in range(4)]
    qrows, krows, vrows = [], [], []
    for h in heads:
        base = h * 3 * DH
        qrows.append(np.arange(base, base + DH))
        krows.append(np.arange(base + DH, base + 2 * DH))
        vrows.append(np.arange(base + 2 * DH, base + 3 * DH))
    # m-chunks: [qP0, kP0, qP1, kP1]; each pair chunk = [head_even | head_odd]
    qk_order = np.concatenate(
        [qrows[0], qrows[1], krows[0], krows[1], qrows[2], qrows[3], krows[2], krows[3]]
    )
    wqk = (w_qkv[qk_order].T * SCALE).astype(np.float16)
    bqk = np.ascontiguousarray((b_qkv[qk_order] * SCALE).reshape(4, 128).T)
    # v weights: [C, 4 heads, 64]; ones column appended on-chip via memset
    vrows_cat = np.concatenate(vrows)
    wv = np.ascontiguousarray(w_qkv[vrows_cat].T).astype(np.float16)
    att_cols = np.concatenate([np.arange(h * DH, (h + 1) * DH) for h in heads])
    wp = (w_proj[:, att_cols].T).astype(np.float16)
    # v-bias folded into projection bias: wp.T @ bv is this head-group's
    # constant contribution to every output column (exact for any b_qkv).
    bv = b_qkv[vrows_cat]  # (256,)
    bp_fold = w_proj[:, att_cols] @ bv  # (512,)
    if hg == 0:
        bp = np.ascontiguousarray((b_proj + bp_fold).reshape(4, 128).T)
        rs = np.ones((128, 1), np.float32)
    else:
        bp = np.ascontiguousarray(bp_fold.reshape(4, 128).T)
        rs = np.zeros((128, 1), np.float32)
    gamma = np.ascontiguousarray(gn_gamma.reshape(4, 128).T)
    beta = np.ascontiguousarray(gn_beta.reshape(4, 128).T)
    ones16 = np.zeros((128, 8), np.float32)
    for g in range(8):
        ones16[g * 16 : (g + 1) * 16, g] = 1.0 / GS
    smalls = np.concatenate([bqk, bp, rs, gamma, beta, ones16], axis=1)
    return dict(wqk=wqk, wv=wv, wp=wp, smalls=smalls.astype(np.float32))


def kernel(x, gn_gamma, gn_beta, w_qkv, b_qkv, w_proj, b_proj, _trace=False):
    x = np.asarray(x, np.float32)
    gn_gamma = np.asarray(gn_gamma, np.float32)
    gn_beta = np.asarray(gn_beta, np.float32)
    w_qkv = np.asarray(w_qkv, np.float32)
    b_qkv = np.asarray(b_qkv, np.float32)
    w_proj = np.asarray(w_proj, np.float32)
    b_proj = np.asarray(b_proj, np.float32)

    if "nc" not in _CACHE:
        _CACHE["nc"] = build_program()
    nc = _CACHE["nc"]

    expander, sel = _consts()
    hg_consts = [
        _core_weights(hg, w_qkv, b_qkv, w_proj, b_proj, gn_gamma, gn_beta)
        for hg in range(2)
    ]
    in_maps = []
    for core in range(8):
        b, hg = core // 2, core % 2
        m = dict(hg_consts[hg])
        m["x"] = np.ascontiguousarray(x[b])
        m["expander"] = expander
        m["sel"] = sel
        in_maps.append(m)

    res = run_bass_kernel_spmd(
        nc, in_maps, core_ids=list(range(8)), trace=_trace
    )
    y = np.empty((B, C, T), np.float32)
    for b in range(B):
        y[b] = res.results[2 * b]["y"] + res.results[2 * b + 1]["y"]
    if _trace:
        _CACHE["last_results"] = res
    return y
